# revision 100
# baseline (speedup 1.0000x reference)
"""Trainium2 Bass kernel for nn_Attention_55894704390617 (v3).

Dense transformer attention block:
  xn = LN(x) ; q,k,v = xn @ wq/wk/wv ; q,k = headLN(q),headLN(k)
  out = softmax(q k^T / sqrt(dh)) v @ wo

Sharding over 8 NeuronCores: 2 (batch) x 4 (head groups of 8 heads).
Each core computes a partial output; the host sums 4 partials per batch.

v4 scheduling changes over v2 (487us -> 426.5us):
  - every hp-loop's q-prefetch fast-finishes (9 qop pieces/jt, finish
    at jt==5 with immediate DVE applies + PE transposes): the next
    chunk's first scores no longer stall ~6us on the last qT block made
    at the boundary, and the loop-end DVE stays clear for the avT chain.
    Sweep results: finish jt==11 -> 452us, jt==8 -> 440, jt==5 -> 429.3,
    jt==4 or rate-13 -> worse. Slots (7,9,11,14) hp0 / (5,8,11,14) rest.
  - KNOWN ~20us LIMIT at the phase-A->B seam (diagnosed, not fixable
    cheaply): first scores wait an ACT EventSemaphore gated on DVE>=579
    = vp14's finish_v (~125us) -- the score pool banks are psA's and V's
    rotation puts its last tiles on every bank. Bank steering fails
    (vp14/15 land wherever sc goes); V-in-tpA slows V ~7us (depth-2
    cadence); the fix needs V on dedicated banks + K regrouped + PE
    stream interleaving, a full-phase-merge redesign.
  - Q0's qT transposes ride the HWDGE (idle by then) instead of tpA+ACT
    copies; tail psum drains alternate DVE/ACT; the tail's last row
    stores as quarters; oproj slots (7,9,11,14) for hp0.
  - the tail's 16 out-projections rotate over FOUR psum banks
    (op/qp/sc/av tags -- all free after the last chunk) instead of
    ping-ponging two, hiding the drain latency (-2.8us). The same trick
    applied to IN-LOOP oproj slots regresses ~12us (qp-bank conflict
    with the next loop's q-prefetch) -- don't.

v3 scheduling changes over v2:
  - mu correction row is built with a tiny PE transpose of the mu column
    (no DRAM roundtrip, no ACT-queue DMAs); stats loads are batched
    [128,2,D] and run one K-quarter ahead so the DVE stats chain is never
    on the PE critical path.
  - ALL phase-B transposes (qT, avT) run on PE into the idle qp/op psum
    banks (tile-tag reuse) with DVE copies out -- the serial 625ns-per-DMA
    HWDGE descriptor unit is out of the attention loop entirely.
  - the av drain + reciprocal + normalize chain is all-DVE; the deferred
    q-LN applies run on DVE after the normalize so the avT path is never
    stuck behind them; K-LN applies are split ACT(Identity, fused
    (x*rstd - mu*rstd))/Pool so kl halves are produced in parallel.
  - osb output stores are issued as half rows on the SP queue so avT/qT
    data never queues behind a 1.5us DMA_ENGINES hold; oproj filler slots
    sit at jt 5/8/11/14 (11-14 for hp0, whose avT dependency lands late).
  - psum->sbuf drain copies (kT blocks, mu rows, Q0 tpq) ride the ACT
    queue, which is idle in phase A.

Key design points (vs the earlier baseline):
  - QKV projections run as fp8e4 DoubleRow matmuls (256-K per instruction,
    0.5 cycles/row): x^T and the norm_w-folded weights are shipped from the
    host as hi/lo fp8 pairs (x scaled by 4, w by 64 to stay in e4m3 range),
    and the three first-order product chains (hh, hl, lh) accumulate into
    one PSUM group -> bf16-class accuracy at ~4x the PE rate.
    The x-LN rstd is NOT applied for Q/K (head-LN is scale-invariant); the
    net 256x scale washes out in head-LN and is folded into V's rstd.
  - The x-LN mean correction stays a K=1 bf16 matmul row (mu as a bf16 row
    via a DRAM round-trip, negc = -colsum(w) shipped from host).
  - Attention AV uses attn as the STATIONARY operand and natural v_ext
    (with a ones column) as the moving operand: out [128 q, 65] costs 65
    cycles instead of 512, and the softmax denominator lands as a PSUM
    column -> normalization is a per-partition tensor_scalar, no broadcast
    matmuls.
  - exp is batched over both heads of a pair ([128, 1024] over a 2-bank
    PSUM AP) to amortize the ACT access bubble; rsqrt uses Sqrt+reciprocal
    (AF.Ln crashes at runtime on this stack -- do not use it).
  - kT/Q0 transposes run on PE (psum pool) + DVE copies; qT/avT use
    dma_start_transpose (xbar).
  - Elementwise work is spread: ACT = exp + rstd only; DVE = stats,
    PSUM-reading applies, normalize; Pool(gpsimd) = SBUF-only applies
    (Pool cannot touch PSUM).
"""

import numpy as np

S = 2048
D = 2048
H_LOC = 8
DH = 64
M_LOC = H_LOC * DH     # 512
N_S = S // 128         # 16 s-tiles
N_TT = D // 256        # 8 double-k tiles
N_SC = S // 512        # 4 query chunks
N_HP = H_LOC // 2      # 4 head pairs
EPS = 1e-5

_COMPILED = {}

import os
MU_SIMPLE = os.environ.get("KN_MU_SIMPLE", "0") == "1"
EXP1BANK = os.environ.get("KN_EXP1BANK", "0") == "1"
NO_POOL = os.environ.get("KN_NO_POOL", "0") == "1"
PHASE_A_ONLY = os.environ.get("KN_PHASE_A_ONLY", "0") == "1"
ONLY = os.environ.get("KN_ONLY", "")



def _build():
    from concourse._compat import axon_active
    axon_active()
    import concourse.bacc as bacc
    import concourse.mybir as mybir
    import concourse.tile as tile
    from concourse.bass import AP
    from contextlib import ExitStack
    import math

    F32 = mybir.dt.float32
    BF16 = mybir.dt.bfloat16
    FP8 = mybir.dt.float8e4
    AF = mybir.ActivationFunctionType
    OP = mybir.AluOpType
    DR = mybir.MatmulPerfMode.DoubleRow
    NL256 = -math.log(256.0)

    nc = bacc.Bacc(None, target_bir_lowering=False)

    x_nat = nc.dram_tensor("x_nat", [S, D], FP8, kind="ExternalInput")
    x2h = nc.dram_tensor("x2h", [N_TT, 128, 2, S], FP8, kind="ExternalInput")
    x2l = nc.dram_tensor("x2l", [N_TT, 128, 2, S], FP8, kind="ExternalInput")
    w2q = nc.dram_tensor("w2q", [N_TT, 128, 2, 2 * M_LOC], FP8, kind="ExternalInput")
    w2k = nc.dram_tensor("w2k", [N_TT, 128, 2, 2 * M_LOC], FP8, kind="ExternalInput")
    w2v = nc.dram_tensor("w2v", [N_TT, 128, 2, 2 * M_LOC], FP8, kind="ExternalInput")
    wo = nc.dram_tensor("wo", [M_LOC, D], BF16, kind="ExternalInput")
    negc = nc.dram_tensor("negc", [3, M_LOC], BF16, kind="ExternalInput")
    g2w = nc.dram_tensor("g2w", [DH], F32, kind="ExternalInput")
    out = nc.dram_tensor("out", [S, D], BF16, kind="ExternalOutput")

    gp = nc.vector if NO_POOL else nc.gpsimd
    with tile.TileContext(nc) as tc:
        es = ExitStack()
        consts = es.enter_context(tc.tile_pool(name="consts", bufs=1))
        x2_pool = es.enter_context(tc.tile_pool(name="x2", bufs=1))
        wq_pool = es.enter_context(tc.tile_pool(name="wqp", bufs=1))
        kT_pool = es.enter_context(tc.tile_pool(name="kT", bufs=1))
        vext_pool = es.enter_context(tc.tile_pool(name="vext", bufs=1))
        qT_pool = es.enter_context(tc.tile_pool(name="qT", bufs=1))
        avT_pool = es.enter_context(tc.tile_pool(name="avT", bufs=1))
        mu_pool = es.enter_context(tc.tile_pool(name="mu", bufs=1))

        from concourse.masks import make_identity
        ident = consts.tile([128, 128], BF16, name="ident")
        make_identity(nc, ident)
        eps_t = consts.tile([128, 1], F32, name="eps_t")
        nc.vector.memset(eps_t, EPS)
        nl256_t = consts.tile([128, 1], F32, name="nl256")
        nc.vector.memset(nl256_t, NL256)
        eps64k_t = consts.tile([128, 1], F32, name="eps64k")
        nc.vector.memset(eps64k_t, EPS * 65536.0)
        g2_rep = consts.tile([128, H_LOC, DH], F32, name="g2_rep")
        negc_sb = [consts.tile([1, M_LOC], BF16, name=f"negc{w}")
                   for w in range(3)]

        def emit_const_dmas():
            bsrc = AP(tensor=g2w[:].tensor, offset=g2w[:].offset,
                      ap=[[0, 128], [0, H_LOC], [1, DH]])
            nc.sync.dma_start(out=g2_rep, in_=bsrc)
            for w in range(3):
                nc.sync.dma_start(out=negc_sb[w], in_=negc[w:w + 1, :])

        xh_t = [x2_pool.tile([128, 2, S], FP8, name=f"xh{t}") for t in range(N_TT)]
        xl_t = [x2_pool.tile([128, 2, S], FP8, name=f"xl{t}") for t in range(N_TT)]
        wq_t = [wq_pool.tile([128, 2, 2 * M_LOC], FP8, name=f"wq{t}")
                for t in range(N_TT)]
        kT = [kT_pool.tile([128, S], BF16, name=f"kT{hp}") for hp in range(N_HP)]
        v_ext = [vext_pool.tile([128, H_LOC, DH + 1], BF16, name=f"vext{st}")
                 for st in range(N_S)]
        qT = [[qT_pool.tile([128, 512], BF16, name=f"qT{p}_{hp}")
               for hp in range(N_HP)] for p in range(2)]
        avT = [[avT_pool.tile([128, 512], BF16, name=f"avT{p}_{hp}")
                for hp in range(N_HP)] for p in range(2)]
        # mu4t[st] = 4*mu of s-tile st as a [1,128] stationary row
        # (PE transpose of the mu column -- no DRAM roundtrip)
        mu4t = [mu_pool.tile([1, 128], BF16, name=f"mu4t_{st}")
                for st in range(N_S)]
        rstdv = [mu_pool.tile([128, 1], F32, name=f"rstdv{st}") for st in range(N_S)]
        mucol = [mu_pool.tile([128, 1], BF16, name=f"mucol{st}")
                 for st in range(N_S)]

        def ssl(st):
            return slice(st * 128, (st + 1) * 128)

        # ones columns of v_ext (disjoint from the value columns)
        for st in range(N_S):
            gp.memset(v_ext[st][:, :, DH:DH + 1], 1.0)

        # ============ phase A ============
        with ExitStack() as ph:
            wkv_pool = ph.enter_context(tc.tile_pool(name="wkv", bufs=1))
            xstage = ph.enter_context(tc.tile_pool(name="xstage", bufs=2))
            scrA = ph.enter_context(tc.tile_pool(name="scrA", bufs=4))
            natst = ph.enter_context(tc.tile_pool(name="natst", bufs=2))
            lnst = ph.enter_context(tc.tile_pool(name="lnst", bufs=2))
            klst = ph.enter_context(tc.tile_pool(name="klst", bufs=6))
            rsgA = ph.enter_context(tc.tile_pool(name="rsgA", bufs=2))
            psA = ph.enter_context(tc.tile_pool(name="psA", bufs=6, space="PSUM"))
            tpA = ph.enter_context(tc.tile_pool(name="tpA", bufs=2, space="PSUM"))

            wk_t = [wkv_pool.tile([128, 2, 2 * M_LOC], FP8, name=f"wk{t}")
                    for t in range(N_TT)]
            wv_t = [wkv_pool.tile([128, 2, 2 * M_LOC], FP8, name=f"wv{t}")
                    for t in range(N_TT)]

            _xs4 = {}

            def emit_stats_load(g):
                # one batched [128,2,D] load per s-tile pair on the ACT hwdge
                # queue (never head-of-line blocks the SP bulk-load stream;
                # batching keeps the ACT sequencer free for psum drains)
                xs2 = xstage.tile([128, 2, D], FP8, tag="xst")
                nc.scalar.dma_start(
                    out=xs2,
                    in_=x_nat[g * 256:(g + 1) * 256, :].rearrange(
                        "(j p) d -> p j d", p=128))
                _xs4[g] = xs2

            def emit_stats(st):
                xg = _xs4[st // 2][:, st % 2, :].rearrange(
                    "p (n f) -> p n f", f=512)
                bn = scrA.tile([128, 4, 6], F32, tag="bn")
                for sg in range(4):
                    nc.vector.bn_stats(out=bn[:, sg, :], in_=xg[:, sg, :])
                mv = scrA.tile([128, 2], F32, tag="mv")
                nc.vector.bn_aggr(out=mv, in_=bn)
                nc.vector.tensor_scalar_mul(
                    out=mucol[st], in0=mv[:, 0:1], scalar1=4.0)
                el = scrA.tile([128, 1], F32, tag="el")
                nc.scalar.activation(out=el, in_=mv[:, 1:2], func=AF.Sqrt,
                                     bias=eps64k_t, scale=65536.0)
                nc.vector.reciprocal(rstdv[st], el)

            def emit_mu_tp(st):
                # transpose the mu column on PE: [128,1] -> [1,128].
                # Emitted at quarter-finish time so the DVE stats chain is
                # long done and the in-order PE queue never stalls on it.
                mt = tpA.tile([1, 128], BF16, tag="tp")
                nc.tensor.transpose(mt[:, :], mucol[st], ident[:, :])
                nc.scalar.copy(out=mu4t[st], in_=mt)

            def emit_corr(ps, st, w):
                nc.tensor.matmul(
                    ps[:, :], mu4t[st][:, :],
                    negc_sb[w][:, :], start=False, stop=True)

            def emit_chain_tt(ps, wt, tt, st, first):
                sl = ssl(st)
                nc.tensor.matmul(ps[:, :], xh_t[tt][:, :, sl],
                                 wt[:, :, 0:M_LOC],
                                 start=first, stop=False, perf_mode=DR)
                nc.tensor.matmul(ps[:, :], xh_t[tt][:, :, sl],
                                 wt[:, :, M_LOC:2 * M_LOC],
                                 start=False, stop=False, perf_mode=DR)
                nc.tensor.matmul(ps[:, :], xl_t[tt][:, :, sl],
                                 wt[:, :, 0:M_LOC],
                                 start=False, stop=False, perf_mode=DR)

            def emit_headln_stats(src, scr_pool, newton=False):
                # per-head bn stats; rstd8 = 1/sqrt(var256 + eps*65536)
                # (i.e. rstd_true/256).  newton=True computes rstd_true
                # entirely on DVE (no ACT Sqrt -> no exp-table swap in the
                # attention stream); callers fold the missing /256 into g2.
                bn8 = scr_pool.tile([128, H_LOC, 6], F32, tag="bn8")
                mv8 = scr_pool.tile([128, H_LOC, 2], F32, tag="mv8")
                for h in range(H_LOC):
                    nc.vector.bn_stats(out=bn8[:, h, :],
                                       in_=src[:, h * DH:(h + 1) * DH])
                    nc.vector.bn_aggr(out=mv8[:, h, :], in_=bn8[:, h, :])
                rstd8 = scr_pool.tile([128, H_LOC], F32, tag="rstd8")
                if not newton:
                    el8 = scr_pool.tile([128, H_LOC], F32, tag="el8")
                    nc.scalar.activation(out=el8, in_=mv8[:, :, 1],
                                         func=AF.Sqrt, bias=eps64k_t,
                                         scale=1.0)
                    nc.vector.reciprocal(rstd8, el8)
                    return mv8, rstd8
                u = scr_pool.tile([128, H_LOC], F32, tag="nu")
                nc.vector.tensor_scalar(out=u, in0=mv8[:, :, 1],
                                        scalar1=1.0 / 65536.0, scalar2=EPS,
                                        op0=OP.mult, op1=OP.add)
                us = scr_pool.tile([128, H_LOC], F32, tag="nus")
                nc.vector.tensor_scalar_add(out=us, in0=u, scalar1=0.4)
                nc.vector.reciprocal(rstd8, us)           # y0 = 1/(u+0.4)
                t = scr_pool.tile([128, H_LOC], F32, tag="nt")
                for _ in range(4):                        # y *= 1.5-0.5*u*y^2
                    nc.vector.tensor_mul(out=t, in0=rstd8, in1=rstd8)
                    nc.vector.tensor_mul(out=t, in0=t, in1=u)
                    nc.vector.tensor_scalar(out=t, in0=t, scalar1=-0.5,
                                            scalar2=1.5, op0=OP.mult,
                                            op1=OP.add)
                    nc.vector.tensor_mul(out=rstd8, in0=rstd8, in1=t)
                return mv8, rstd8

            def finish_k_ln(st, ps):
                emit_corr(ps, st, 1)
                # free the psum bank with one ACT copy; stats+apply off psum
                knat = natst.tile([128, M_LOC], F32, tag="knat")
                nc.scalar.copy(out=knat, in_=ps)
                mv8, rstd8 = emit_headln_stats(knat, scrA)
                kl = klst.tile([128, M_LOC], BF16, tag="kl", name=f"kl{st}")
                # split the LN apply across ACT (heads 0-3, fused
                # Identity(x*rstd - mu*rstd)) and Pool (heads 4-7): the two
                # engines produce kl halves in parallel, halving the latency
                # the kl transposes wait on
                nm = scrA.tile([128, H_LOC], F32, tag="nmr")
                nc.vector.tensor_mul(out=nm, in0=mv8[:, :, 0], in1=rstd8)
                nc.vector.tensor_scalar_mul(out=nm, in0=nm, scalar1=-1.0)
                kc = natst.tile([128, M_LOC], BF16, tag="kc")
                for h in range(H_LOC):
                    if h < 3:
                        nc.scalar.activation(
                            out=kl[:, h * DH:(h + 1) * DH],
                            in_=knat[:, h * DH:(h + 1) * DH],
                            func=AF.Identity, bias=nm[:, h:h + 1],
                            scale=rstd8[:, h:h + 1])
                    else:
                        gp.tensor_scalar_sub(
                            out=kc[:, h * DH:(h + 1) * DH],
                            in0=knat[:, h * DH:(h + 1) * DH],
                            scalar1=mv8[:, h, 0:1])
                        gp.tensor_scalar_mul(
                            out=kl[:, h * DH:(h + 1) * DH],
                            in0=kc[:, h * DH:(h + 1) * DH],
                            scalar1=rstd8[:, h:h + 1])
                return kl

            def finish_k_tp(st, kl):
                tp = tpA.tile([128, 4, 128], BF16, tag="tp")
                for b in range(N_HP):
                    nc.tensor.transpose(tp[:, b, :],
                                        kl[:, b * 128:(b + 1) * 128],
                                        ident[:, :])
                for b in range(N_HP):
                    nc.scalar.copy(out=kT[b][:, ssl(st)], in_=tp[:, b, :])

            def finish_v(st, ps):
                emit_corr(ps, st, 2)
                nc.vector.tensor_scalar_mul(
                    out=v_ext[st][:, :, 0:DH],
                    in0=ps.rearrange("p (h d) -> p h d", d=DH),
                    scalar1=rstdv[st])

            def finish_q(st, ps, par, scr_pool, rsg_pool, ln_pool, nat_pool,
                         tp_pool=None):
                # free the qp PSUM bank with one DVE copy, then head-LN with
                # qn*kn folded in, applies on the (idle) Pool engine.
                # Without tp_pool, the Pool applies are DEFERRED: the caller
                # invokes the returned closure after the avbf normalize so
                # the normalize is never stuck behind them in the Pool queue.
                qnat = nat_pool.tile([128, M_LOC], F32, tag="qnat")
                nc.vector.tensor_copy(qnat, ps)
                mv8, rstd8 = emit_headln_stats(qnat, scr_pool, newton=True)

                def emit_applies(eng=None):
                    e = eng or gp
                    rsg = rsg_pool.tile([128, H_LOC, DH], BF16, tag="rsg")
                    for h in range(H_LOC):
                        e.tensor_scalar_mul(
                            out=rsg[:, h, :], in0=g2_rep[:, h, :],
                            scalar1=rstd8[:, h:h + 1])
                    ql = ln_pool.tile([128, M_LOC], BF16, tag="ql")
                    qc = nat_pool.tile([128, M_LOC], BF16, tag="qc")
                    for h in range(H_LOC):
                        e.tensor_scalar_sub(
                            out=qc[:, h * DH:(h + 1) * DH],
                            in0=qnat[:, h * DH:(h + 1) * DH],
                            scalar1=mv8[:, h, 0:1])
                    e.tensor_mul(
                        out=ql, in0=qc,
                        in1=rsg.rearrange("p h d -> p (h d)"))
                    return ql

                if tp_pool is not None:
                    ql = emit_applies()
                    tpq = tp_pool.tile([128, 4, 128], BF16, tag="tp")
                    for b in range(N_HP):
                        nc.tensor.transpose(tpq[:, b, :],
                                            ql[:, b * 128:(b + 1) * 128],
                                            ident[:, :])
                    for b in range(N_HP):
                        nc.scalar.copy(
                            out=qT[par][b][:, (st % 4) * 128:(st % 4 + 1) * 128],
                            in_=tpq[:, b, :])
                else:
                    return emit_applies

            # ---- K projection, contraction-outer in quarters of 4 ----
            kp = {}
            kls = {}
            if ONLY:
                # bisect mode: memset stand-ins for skipped producers
                for st in range(N_S):
                    gp.memset(mu4t[st], 0.01)
                for st in range(N_S):
                    gp.memset(rstdv[st], 1.0)
                for hp in range(N_HP):
                    gp.memset(kT[hp], 0.1)
                for p_ in range(2):
                    for hp in range(N_HP):
                        gp.memset(qT[p_][hp], 0.1)
                for st in range(N_S):
                    gp.memset(v_ext[st][:, :, 0:DH], 0.1)
            if ONLY == "stats":
                for st in range(N_S):
                    if st % 2 == 0:
                        emit_stats_load(st // 2)
                    emit_stats(st)
                    emit_mu_tp(st)
            HS = S // 2
            if ONLY in ("", "k", "kv", "kvq"):
              for quarter in range(4):
                  for tt in range(N_TT):
                      if quarter == 0:
                          nc.sync.dma_start(out=wk_t[tt], in_=w2k[tt, :, :, :])
                          # tt 0 lands in two pieces so the very first K
                          # chain starts ~2.5us sooner on a cold DMA pipe
                          for piece in ((0, 512), (512, HS)) if tt == 0                                   else ((0, HS),):
                              lo, hi = piece
                              nc.sync.dma_start(out=xh_t[tt][:, :, lo:hi],
                                                in_=x2h[tt, :, :, lo:hi])
                              nc.sync.dma_start(out=xl_t[tt][:, :, lo:hi],
                                                in_=x2l[tt, :, :, lo:hi])
                          if tt == 1:
                              emit_const_dmas()
                      # stats run one quarter ahead of their consumers so
                      # the DVE chain is never on the PE critical path; the
                      # first stats load trails the first K-proj loads so
                      # the PE isn't DMA-starved at kernel start
                      if quarter == 0:
                          if tt in (1, 2, 4, 6):
                              emit_stats_load((0, 0, 1, 2, 2, 3, 3, 3)[tt])
                          if tt in (3, 5, 7):
                              g = (tt - 3) // 2
                              emit_stats(2 * g)
                              emit_stats(2 * g + 1)
                      elif quarter == 1:
                          if tt == 0:
                              emit_stats(6)
                              emit_stats(7)
                          if tt in (0, 4):
                              emit_stats_load(4 + tt // 4)
                          if tt in (1, 2, 5, 6):
                              emit_stats(8 + (tt - 1 if tt <= 2 else tt - 3))
                      elif quarter == 2:
                          if tt in (0, 4):
                              emit_stats_load(6 + tt // 4)
                          if tt in (1, 2, 5, 6):
                              emit_stats(12 + (tt - 1 if tt <= 2 else tt - 3))
                      for si in range(4):
                          st = quarter * 4 + si
                          if tt == 0:
                              kp[st] = psA.tile([128, M_LOC], F32, tag="pa",
                                                name=f"kp{st}")
                          emit_chain_tt(kp[st], wk_t[tt], tt, st,
                                        first=(tt == 0))
                  if quarter == 0:
                      # second column halves, then Q/V weights behind them
                      for tt in range(N_TT):
                          nc.sync.dma_start(out=xh_t[tt][:, :, HS:S],
                                            in_=x2h[tt, :, :, HS:S])
                          nc.sync.dma_start(out=xl_t[tt][:, :, HS:S],
                                            in_=x2l[tt, :, :, HS:S])
                      for tt in range(N_TT):
                          nc.sync.dma_start(out=wq_t[tt], in_=w2q[tt, :, :, :])
                  if quarter == 1:
                      for tt in range(N_TT):
                          nc.sync.dma_start(out=wv_t[tt], in_=w2v[tt, :, :, :])
                  for si in range(4):
                      emit_mu_tp(quarter * 4 + si)
                  for si in range(4):
                      st = quarter * 4 + si
                      kls[st] = finish_k_ln(st, kp[st])
                  if quarter > 0:
                      for si in range(4):
                          st = (quarter - 1) * 4 + si
                          finish_k_tp(st, kls.pop(st))

              for si in range(4):
                  st = 12 + si
                  finish_k_tp(st, kls.pop(st))

            # ---- Q projection for chunk 0 (its finish overlaps V).
            # qT0 transposes ride the (idle-by-now) HWDGE so neither tpA
            # nor psA banks are held: psA's banks -- which the phase-B
            # score pool aliases -- free right after these chains, letting
            # chunk-0 sc/exp hoist into the V window ----
            for hp in (range(N_HP) if ONLY in ("", "q", "kvq") else []):
                ps = psA.tile([128, M_LOC], F32, tag="pa", name=f"qp0_{hp}")
                for tt in range(N_TT):
                    emit_chain_tt(ps, wq_t[tt], tt, hp, first=(tt == 0))
                emit_corr(ps, hp, 0)
                q_ap = finish_q(hp, ps, 0, scrA, rsgA, lnst, natst)
                ql0 = q_ap()
                for b in range(N_HP):
                    nc.sync.dma_start_transpose(
                        out=qT[0][b][:, (hp % 4) * 128:(hp % 4 + 1) * 128],
                        in_=ql0[:, b * 128:(b + 1) * 128])

            # ---- V projection, s-outer ----
            for st in (range(N_S) if ONLY in ("", "v", "kv", "kvq") else []):
                ps = psA.tile([128, M_LOC], F32, tag="pa", name=f"vp{st}")
                for tt in range(N_TT):
                    emit_chain_tt(ps, wv_t[tt], tt, st, first=(tt == 0))
                finish_v(st, ps)

        if PHASE_A_ONLY:
            # debug: dump phase-A products and skip attention entirely
            for hp in range(N_HP):
                nc.sync.dma_start(out=out[hp * 128:(hp + 1) * 128, :],
                                  in_=kT[hp])
            for st in range(4):
                nc.sync.dma_start(
                    out=out[(4 + st) * 128:(5 + st) * 128, 0:520],
                    in_=v_ext[st].rearrange("p h d -> p (h d)"))
                nc.sync.dma_start(
                    out=out[(8 + st) * 128:(9 + st) * 128, 0:512],
                    in_=qT[0][st])

        if not PHASE_A_ONLY:
            # ============ phase B: attention ============
            with ExitStack() as ph:
                wo_pool = ph.enter_context(tc.tile_pool(name="wop", bufs=1))
                at_pool = ph.enter_context(tc.tile_pool(name="at", bufs=5))
                avbf_pool = ph.enter_context(tc.tile_pool(name="avbf", bufs=2))
                rden_pool = ph.enter_context(tc.tile_pool(name="rden", bufs=2))
                osb_pool = ph.enter_context(tc.tile_pool(name="osb", bufs=2))
                scrB = ph.enter_context(tc.tile_pool(name="scrB", bufs=4))
                lnstB = ph.enter_context(tc.tile_pool(name="lnstB", bufs=2))
                rsgB = ph.enter_context(tc.tile_pool(name="rsgB", bufs=2))
                qnatB = ph.enter_context(tc.tile_pool(name="qnatB", bufs=2))
                ps_sc = ph.enter_context(tc.tile_pool(name="ps_sc", bufs=2, space="PSUM"))
                ps_av = ph.enter_context(tc.tile_pool(name="ps_av", bufs=1, space="PSUM"))
                ps_qp = ph.enter_context(tc.tile_pool(name="ps_qp", bufs=1, space="PSUM"))
                ps_op = ph.enter_context(tc.tile_pool(name="ps_op", bufs=1, space="PSUM"))

                wo_bf = [wo_pool.tile([128, D], BF16, name=f"wo{mt}")
                         for mt in range(N_HP)]
                for mt in range(N_HP):
                    nc.sync.dma_start(out=wo_bf[mt], in_=wo[ssl(mt), :])

                def emit_oproj(ics, hp, do, osb_t, pool=None, act_copy=False):
                    st = 4 * ics + hp
                    lsl = slice((st % 4) * 128, (st % 4 + 1) * 128)
                    p_, ptag = pool or (ps_op, "op")
                    op = p_.tile([128, 512], F32, tag=ptag,
                                 name=f"op{st}_{do}")
                    for mt in range(N_HP):
                        nc.tensor.matmul(op[:, :], avT[ics % 2][mt][:, lsl],
                                         wo_bf[mt][:, do * 512:(do + 1) * 512],
                                         start=(mt == 0), stop=(mt == N_HP - 1))
                    if act_copy:
                        # tail only: ACT is idle there, so alternating the
                        # psum drains between DVE and ACT halves the copy
                        # stream that gates the final stores
                        nc.scalar.copy(out=osb_t[:, do * 512:(do + 1) * 512],
                                       in_=op)
                    else:
                        nc.vector.tensor_copy(
                            osb_t[:, do * 512:(do + 1) * 512], op)
                    # half-row stores on the DVE queue: smaller DMA_ENGINES
                    # holds so the next hp's avT transposes aren't stuck
                    # behind a full-row store
                    if do in (1, 3):
                        if ics == 3 and hp == 3:
                            # tail's last row: two quarter stores pipeline
                            # the final DMA chain ahead of the end barrier
                            for qd in (do - 1, do):
                                nc.sync.dma_start(
                                    out=out[ssl(st), qd * 512:(qd + 1) * 512],
                                    in_=osb_t[:, qd * 512:(qd + 1) * 512])
                        else:
                            nc.sync.dma_start(
                                out=out[ssl(st), (do - 1) * 512:(do + 1) * 512],
                                in_=osb_t[:, (do - 1) * 512:(do + 1) * 512])

                for ic in range(N_SC):
                    par = ic % 2
                    for hp in range(N_HP):
                        qst = 4 * (ic + 1) + hp if ic + 1 < N_SC else None
                        qp = None
                        qops = []
                        if qst is not None:
                            qp = ps_qp.tile([128, M_LOC], F32, tag="qp",
                                            name=f"qp{qst}")
                            for tt in range(N_TT):
                                for ci in range(3):
                                    qops.append((tt, ci))
                            qops.append(("corr", 0))
                        av = ps_av.tile([128, 2, 512], F32, tag="av",
                                        name=f"av{ic}_{hp}")
                        osb_t = None
                        if ic > 0:
                            osb_t = osb_pool.tile([128, D], BF16, tag="osb",
                                                  name=f"osb{ic}_{hp}")
                        at_prev = None

                        def emit_qop(piece):
                            tt, ci = piece
                            if tt == "corr":
                                emit_corr(qp, qst, 0)
                                return
                            sl = ssl(qst)
                            xt = xh_t[tt] if ci < 2 else xl_t[tt]
                            ws = 0 if ci != 1 else M_LOC
                            nc.tensor.matmul(qp[:, :], xt[:, :, sl],
                                             wq_t[tt][:, :, ws:ws + M_LOC],
                                             start=(tt == 0 and ci == 0),
                                             stop=False, perf_mode=DR)

                        def emit_av(at_t, jt):
                            # start=True clears has_written for the WHOLE bank:
                            # only the first write per bank (g==0) may carry it.
                            for hs in range(2):
                                for g in range(4):
                                    nc.tensor.matmul(
                                        av[:, hs, g * 65:(g + 1) * 65],
                                        at_t[:, hs, g * 128:(g + 1) * 128],
                                        v_ext[jt][:, 2 * hp + hs, :],
                                        start=(jt == 0 and g == 0),
                                        stop=(jt == N_S - 1 and g == 3),
                                        skip_group_check=True)

                        for jt in range(N_S):
                            jsl = ssl(jt)
                            sc = ps_sc.tile([128, 2, 512], F32, tag="sc")
                            for hs in range(2):
                                psl = slice(hs * DH, (hs + 1) * DH)
                                nc.tensor.matmul(sc[:, hs, :], kT[hp][psl, jsl],
                                                 qT[par][hp][psl, :],
                                                 start=True, stop=True)
                            at_t = at_pool.tile([128, 2, 512], BF16, tag="at")
                            if EXP1BANK:
                                for hs in range(2):
                                    nc.scalar.activation(out=at_t[:, hs, :],
                                                         in_=sc[:, hs, :],
                                                         func=AF.Exp, scale=0.125)
                            else:
                                nc.scalar.activation(out=at_t, in_=sc,
                                                     func=AF.Exp, scale=0.125)
                            # exp(jt-1)-independent PE work goes BEFORE av(jt-1)
                            # so the PE never idles waiting for the activation
                            if jt >= 2:
                                # fast consumption: the q-prefetch fully
                                # lands (incl. qT copies) by ~jt 10, keeping
                                # the end-of-loop DVE clear for the avT
                                # drain/normalize chain
                                for _ in range(9):
                                    if qops:
                                        emit_qop(qops.pop(0))
                            # oproj slots at jt 5/8/11/14 (not 3/7/11/15):
                            # the first slot must trail the previous hp's
                            # avT dma transposes by ~3us or PE stalls on
                            # the DMAHW sem (transposes queue behind osb
                            # stores on the shared DMA engines)
                            slots = (7, 9, 11, 14) if hp == 0 else (5, 8, 11, 14)
                            if ic > 0 and jt in slots:
                                emit_oproj(ic - 1, hp, slots.index(jt), osb_t)
                            if at_prev is not None:
                                emit_av(at_prev, jt - 1)
                            at_prev = at_t
                            if jt == 5 and qst is not None:
                                while qops:
                                    emit_qop(qops.pop(0))
                                q_apply = finish_q(qst, qp, 1 - par, scrB,
                                                   rsgB, lnstB, qnatB)
                                # immediate applies + transposes: qT done
                                # mid-loop, DVE free at the loop end
                                ql_n = q_apply(nc.vector)
                                tpq = ps_qp.tile([128, 4, 128], BF16,
                                                 tag="qp",
                                                 name=f"tpq{qst}")
                                for b in range(N_HP):
                                    nc.tensor.transpose(
                                        tpq[:, b, :],
                                        ql_n[:, b * 128:(b + 1) * 128],
                                        ident[:, :])
                                for b in range(N_HP):
                                    nc.vector.tensor_copy(
                                        qT[1 - par][b][:, (qst % 4) * 128:
                                                       (qst % 4 + 1) * 128],
                                        tpq[:, b, :])
                        emit_av(at_prev, N_S - 1)

                        # drain the av bank with ONE DVE copy, then keep the
                        # whole recip+normalize chain on DVE: no Pool hop, so
                        # the avT transposes start ~2us after av15 instead of
                        # queuing behind unrelated Pool work
                        avsb = rden_pool.tile([128, 2, 4, 65], F32, tag="avsb")
                        nc.vector.tensor_copy(
                            avsb,
                            av[:, :, 0:260].rearrange("p h (g c) -> p h g c",
                                                      c=65))
                        rden = rden_pool.tile([128, 2, 4], F32, tag="rden")
                        for hs in range(2):
                            nc.vector.reciprocal(rden[:, hs, :],
                                                 avsb[:, hs, :, DH:DH + 1])
                        avbf = avbf_pool.tile([128, 512], BF16, tag="avbf")
                        for g in range(4):
                            for hs in range(2):
                                nc.vector.tensor_scalar_mul(
                                    out=avbf[:, g * 128 + hs * DH:
                                             g * 128 + (hs + 1) * DH],
                                    in0=avsb[:, hs, g, 0:DH],
                                    scalar1=rden[:, hs, g:g + 1])
                        # avT transposes on PE into the (drained) op bank +
                        # DVE copies out: no HWDGE, ~3us lower latency, so
                        # the next chunk's oproj never stalls on avT
                        tpv = ps_op.tile([128, 4, 128], BF16, tag="op",
                                         name=f"tpv{ic}_{hp}")
                        for g in range(4):
                            nc.tensor.transpose(tpv[:, g, :],
                                                avbf[:, g * 128:(g + 1) * 128],
                                                ident[:, :])
                        for g in range(4):
                            nc.vector.tensor_copy(
                                avT[par][hp][:, g * 128:(g + 1) * 128],
                                tpv[:, g, :])


                # tail: out-projection of the last chunk, ping-ponging
                # between the op bank and the (now idle) qp bank
                for hp in range(N_HP):
                    osb_t = osb_pool.tile([128, D], BF16, tag="osb",
                                          name=f"osbT{hp}")
                    for do in range(4):
                        emit_oproj(3, hp, do, osb_t,
                                   pool=((ps_op, "op"), (ps_qp, "qp"),
                                         (ps_sc, "sc"), (ps_av, "av"))[
                                             (hp * 4 + do) % 4],
                                   act_copy=bool(do % 2))
        es.close()

    nc.compile()
    return nc


def _get_nc():
    if "nc" not in _COMPILED:
        _COMPILED["nc"] = _build()
    return _COMPILED["nc"]


def _split_fp8(a):
    import ml_dtypes
    hi = a.astype(ml_dtypes.float8_e4m3)
    lo = (a - hi.astype(np.float32)).astype(ml_dtypes.float8_e4m3)
    return hi, lo


def _dr_layout(a, free):
    # [D, free] -> [N_TT, 128, 2, free] with row d = 256*tt + 128*ks + p
    return np.ascontiguousarray(
        a.reshape(N_TT, 2, 128, free).transpose(0, 2, 1, 3))


def kernel(x, norm_w, wq, wk, wv, qn_w, kn_w, wo):
    import ml_dtypes
    from concourse.bass_utils import run_bass_kernel_spmd

    BF = ml_dtypes.bfloat16
    x = np.asarray(x, dtype=np.float32)
    norm_w = np.asarray(norm_w, dtype=np.float32)
    wq = np.asarray(wq, dtype=np.float32)
    wk = np.asarray(wk, dtype=np.float32)
    wv = np.asarray(wv, dtype=np.float32)
    qn_w = np.asarray(qn_w, dtype=np.float32)
    kn_w = np.asarray(kn_w, dtype=np.float32)
    wo = np.asarray(wo, dtype=np.float32)
    B = x.shape[0]

    nc = _get_nc()

    xprep = {}
    for b in range(B):
        xt4 = np.ascontiguousarray(x[b].T) * 4.0
        h, l = _split_fp8(xt4)
        xprep[b] = (_dr_layout(h, S), _dr_layout(l, S),
                    x[b].astype(ml_dtypes.float8_e4m3))

    g2 = (qn_w * kn_w / 256.0).astype(np.float32)
    in_maps = []
    for c in range(8):
        b, g = c // 4, c % 4
        ms = slice(g * M_LOC, (g + 1) * M_LOC)

        def wprep(w):
            wg = (norm_w[:, None] * w[:, ms]) * 64.0
            h, l = _split_fp8(wg)
            comb = np.concatenate(
                [_dr_layout(h, M_LOC), _dr_layout(l, M_LOC)], axis=3)
            neg = -(h.astype(np.float32) + l.astype(np.float32)).sum(0)
            return np.ascontiguousarray(comb), neg

        w2q_a, nq = wprep(wq)
        w2k_a, nk = wprep(wk)
        w2v_a, nv = wprep(wv)
        xh, xl, xnat = xprep[b]
        in_maps.append({
            "x_nat": xnat,
            "x2h": xh,
            "x2l": xl,
            "w2q": w2q_a,
            "w2k": w2k_a,
            "w2v": w2v_a,
            "wo": np.ascontiguousarray(wo[ms, :]).astype(BF),
            "negc": np.stack([nq, nk, nv]).astype(BF),
            "g2w": g2,
        })
    res = run_bass_kernel_spmd(nc, in_maps, core_ids=list(range(8)))
    outp = np.zeros((B, S, D), dtype=np.float32)
    for c in range(8):
        outp[c // 4] += np.asarray(res.results[c]["out"]).astype(np.float32)
    return outp

